# revision 35
# baseline (speedup 1.0000x reference)
"""Trainium2 Bass kernel for top-2 MoE routing (nn_MoE_29291676959130).

Strategy: expert-parallel across the 8 NeuronCores (1 expert per core).
  host (shard step) : gating matmul in float64 -> top-2 indices + softmax
                      combine weights; gather each expert's routed tokens,
                      pre-scaled by their combine weight.
  device (per core) : Y = (w*x) @ W_e^T for its <=CAP routed tokens, as a
                      float16 tiled matmul (K=2048 contraction accumulated
                      in fp32 PSUM).
  host (unshard)    : scatter-add the two expert contributions per token,
                      plus the (tiny) combine-weighted bias term.

Shapes (hardcoded): B=4096 tokens, D=2048, H=4096, E=8 experts, top-2.
"""

import numpy as np

import concourse.bass as bass
import concourse.tile as tile
from concourse import bacc, mybir
from concourse.bass_utils import run_bass_kernel_spmd
from concourse.tile_rust import add_dep_helper

B, D, H, E, TOPK = 4096, 2048, 4096, 8, 2
P = 128
KT = D // P          # 16 k-tiles over the contraction dim
NFREE = 512          # PSUM bank free-dim (fp32)
NT = H // NFREE      # 8 n-tiles

_NC_CACHE: dict[int, object] = {}

# Set TRACE=True (e.g. from a test harness) to profile the device kernel;
# the BassKernelResults of the last run lands in LAST_RESULTS.
TRACE = False
LAST_RESULTS = None


XCHUNK = 2           # k-tiles per token/weight DMA chunk


def _build(cap: int):
    """Per-core program: out[cap,H] = xt.T @ wt  (float16 matmuls).

    K=2048 contraction accumulated in fp32 PSUM over 16 k-tile matmuls.
    Two n-tiles are paired per stationary token tile so each PE weight
    load feeds two matmuls -- this keeps the matmul stream at its
    213-216ns/matmul floor (single-n chains run ~259ns). The first
    n-pair's weight slices and the token block arrive as interleaved
    per-k chunk tiles so the first matmuls start ~equal with the DMA
    stream. (wrow/brow/zrow are vestigial and unused by the math.)
    """
    fmm = mybir.dt.float16
    f32 = mybir.dt.float32
    nc = bacc.Bacc("TRN2", target_bir_lowering=False, debug=False, num_devices=E)
    xt = nc.dram_tensor("xt", [D, cap], fmm, kind="ExternalInput").ap()
    wt = nc.dram_tensor("wt", [D, H], fmm, kind="ExternalInput").ap()
    MT = cap // P
    wrow = nc.dram_tensor("wrow", [1, cap], fmm, kind="ExternalInput").ap()
    brow = nc.dram_tensor("brow", [1, H], fmm, kind="ExternalInput").ap()
    out = nc.dram_tensor("out", [cap, H], f32, kind="ExternalOutput").ap()

    with tile.TileContext(nc) as tc:
        with (
            tc.tile_pool(name="xpool", bufs=1) as xpool,
            tc.tile_pool(name="cpool", bufs=1) as cpool,
            tc.tile_pool(name="wpool", bufs=6) as wpool,
            tc.tile_pool(name="opool", bufs=6) as opool,
            tc.tile_pool(name="pspool", bufs=6, space="PSUM") as pspool,
        ):
            xt_r = xt.rearrange("(kt p) c -> p kt c", p=P)
            wt_r = wt.rearrange("(kt p) f -> p kt f", p=P)
            # chunk boundaries over the k-tiles: the first two chunks are a
            # single k-tile so the first matmul's critical DMA path is small
            bounds = [0, 1, 2]
            while bounds[-1] < KT:
                bounds.append(min(KT, bounds[-1] + XCHUNK))
            NCH = len(bounds) - 1
            cof = []
            for c in range(NCH):
                for kk in range(bounds[c + 1] - bounds[c]):
                    cof.append((c, kk))

            # First n-pair's weight slices arrive as per-k chunks,
            # interleaved with the token chunks, so the first matmuls only
            # wait for ~2MB instead of the whole resident set.
            w0c, w1c, xc = [], [], []
            for i in range(NCH):
                lo, hi = bounds[i], bounds[i + 1]
                sz = hi - lo
                tw0 = cpool.tile([P, sz, NFREE], fmm,
                                 name=f"w0c{i}", tag=f"w0c{i}")
                nc.sync.dma_start(tw0[:], wt_r[:, lo:hi, 0:NFREE])
                w0c.append(tw0)
                tx = xpool.tile([P, sz, cap], fmm, name=f"xc{i}", tag=f"xc{i}")
                nc.sync.dma_start(tx[:], xt_r[:, lo:hi, :])
                xc.append(tx)
                tw1 = cpool.tile([P, sz, NFREE], fmm,
                                 name=f"w1c{i}", tag=f"w1c{i}")
                nc.sync.dma_start(tw1[:], wt_r[:, lo:hi, NFREE:2 * NFREE])
                w1c.append(tw1)

            wr = cpool.tile([1, cap], fmm, name="wr", tag="wr")
            nc.sync.dma_start(wr[:], wrow[:])
            br = cpool.tile([1, H], fmm, name="br", tag="br")
            nc.sync.dma_start(br[:], brow[:])
            zrow = cpool.tile([1, P], fmm, name="zrow", tag="zr")
            nc.gpsimd.memset(zrow[:], 0.0)

            # Two n-tiles share each stationary token tile: each weight load
            # feeds two matmuls (halves the PE weight-load pressure).
            for pr in range(NT // 2):
                n0 = 2 * pr
                if pr == 0:
                    wA = lambda k: w0c[cof[k][0]][:, cof[k][1], :]
                    wB = lambda k: w1c[cof[k][0]][:, cof[k][1], :]
                    # Chunk-progressive start: advance the first four
                    # m-blocks one k-chunk at a time across 8 PSUM banks,
                    # so the PE has ~28us of runnable work while the
                    # initial bulk DMA streams in (instead of one block).
                    M1 = min(4, MT)
                    accA = [pspool.tile([P, NFREE], f32, name=f"accA{m}",
                                        tag="acc0", bufs=4)
                            for m in range(M1)]
                    accB = [pspool.tile([P, NFREE], f32, name=f"accB{m}",
                                        tag="acc1", bufs=4)
                            for m in range(M1)]
                    for c in range(NCH):
                        lo, hi = bounds[c], bounds[c + 1]
                        for m in range(M1):
                            for kk in range(hi - lo):
                                k = lo + kk
                                st = xc[c][:, kk, m * P:(m + 1) * P]
                                nc.tensor.matmul(accA[m][:], st,
                                                 w0c[c][:, kk, :],
                                                 start=(k == 0),
                                                 stop=(k == KT - 1))
                                nc.tensor.matmul(accB[m][:], st,
                                                 w1c[c][:, kk, :],
                                                 start=(k == 0),
                                                 stop=(k == KT - 1))
                    for m in range(M1):
                        ot = opool.tile([P, 2 * NFREE], f32, name="ot")
                        nc.vector.tensor_copy(ot[:, 0:NFREE], accA[m][:])
                        nc.vector.tensor_copy(ot[:, NFREE:2 * NFREE], accB[m][:])
                        nc.sync.dma_start(
                            out[m * P:(m + 1) * P, 0:2 * NFREE], ot[:])
                    mstart = M1
                else:
                    mstart = 0
                    wtA = wpool.tile([P, KT, NFREE], fmm, name="wtile")
                    nc.sync.dma_start(
                        wtA[:], wt_r[:, :, n0 * NFREE:(n0 + 1) * NFREE])
                    wtB = wpool.tile([P, KT, NFREE], fmm, name="wtile")
                    nc.sync.dma_start(
                        wtB[:], wt_r[:, :, (n0 + 1) * NFREE:(n0 + 2) * NFREE])
                    wA = lambda k, t=wtA: t[:, k, :]
                    wB = lambda k, t=wtB: t[:, k, :]
                for m in range(mstart, MT):
                    acc0 = pspool.tile([P, NFREE], f32, name="acc0",
                                       tag="acc0", bufs=4)
                    acc1 = pspool.tile([P, NFREE], f32, name="acc1",
                                       tag="acc1", bufs=4)
                    for k in range(KT):
                        st = xc[cof[k][0]][:, cof[k][1], m * P:(m + 1) * P]
                        nc.tensor.matmul(acc0[:], st, wA(k),
                                         start=(k == 0), stop=(k == KT - 1))
                        nc.tensor.matmul(acc1[:], st, wB(k),
                                         start=(k == 0), stop=(k == KT - 1))
                    ot = opool.tile([P, 2 * NFREE], f32, name="ot")
                    nc.vector.tensor_copy(ot[:, 0:NFREE], acc0[:])
                    nc.vector.tensor_copy(ot[:, NFREE:2 * NFREE], acc1[:])
                    nc.sync.dma_start(
                        out[m * P:(m + 1) * P,
                            n0 * NFREE:(n0 + 2) * NFREE],
                        ot[:])
    nc.compile()
    return nc


def _get_nc(cap: int):
    if cap not in _NC_CACHE:
        _NC_CACHE[cap] = _build(cap)
    return _NC_CACHE[cap]


def _route(x, difficulty_labels, emb, gate_W, gate_b):
    """Gating in float64: returns (topk_idx int32 [B,2], probs f64 [B,2])."""
    x64 = x.astype(np.float64)
    w1 = gate_W[:, :D].astype(np.float64)          # [E, D]
    w2 = gate_W[:, D:].astype(np.float64)          # [E, D]
    table = emb.astype(np.float64) @ w2.T          # [NDIFF, E]
    logits = x64 @ w1.T + table[difficulty_labels] + gate_b.astype(np.float64)
    # jax.lax.top_k order: descending value, ties -> lower index first
    topk_idx = np.argsort(-logits, axis=1, kind="stable")[:, :TOPK]
    topw = np.take_along_axis(logits, topk_idx, axis=1)
    ex = np.exp(topw - topw.max(axis=1, keepdims=True))
    probs = ex / ex.sum(axis=1, keepdims=True)
    return topk_idx.astype(np.int32), probs


def kernel(x, difficulty_labels, emb, gate_W, gate_b, expert_W, expert_b):
    x = np.asarray(x, dtype=np.float32)
    difficulty_labels = np.asarray(difficulty_labels)
    emb = np.asarray(emb, dtype=np.float32)
    gate_W = np.asarray(gate_W, dtype=np.float32)
    gate_b = np.asarray(gate_b, dtype=np.float32)
    expert_W = np.asarray(expert_W, dtype=np.float32)
    expert_b = np.asarray(expert_b, dtype=np.float32)

    topk_idx, probs = _route(x, difficulty_labels, emb, gate_W, gate_b)

    # Per-expert routed token lists + combine weights
    rows_per_e, w_per_e = [], []
    for e in range(E):
        hit = topk_idx == e                         # [B, 2]
        mask = hit.any(axis=1)
        rows = np.nonzero(mask)[0]
        w = np.where(hit[rows, 0], probs[rows, 0], probs[rows, 1])
        rows_per_e.append(rows)
        w_per_e.append(w.astype(np.float32))

    # Pass size: normally one pass (max count ~1100 of cap 1152); the
    # multi-pass split only triggers for pathologically imbalanced routing
    # that would otherwise overflow SBUF.
    MAXCAP = 1408
    maxcnt = max(len(r) for r in rows_per_e)
    npass = max(1, -(-maxcnt // MAXCAP))
    seg = -(-maxcnt // npass)
    cap = max(P, -(-seg // P) * P)
    nc = _get_nc(cap)

    combine = np.zeros((B, E), dtype=np.float32)
    combine[np.arange(B), topk_idx[:, 0]] = probs[:, 0]
    combine[np.arange(B), topk_idx[:, 1]] = probs[:, 1]
    out = (combine @ expert_b).astype(np.float32)

    global LAST_RESULTS
    for s in range(npass):
        in_maps = []
        for e in range(E):
            rows = rows_per_e[e][s * seg:(s + 1) * seg]
            w = w_per_e[e][s * seg:(s + 1) * seg]
            xs = np.zeros((cap, D), dtype=np.float32)
            xs[: len(rows)] = x[rows] * w[:, None]
            wpad = np.zeros((cap,), dtype=np.float32)
            wpad[: len(rows)] = w
            in_maps.append(
                {
                    "xt": xs.T.astype(np.float16),
                    "wt": expert_W[e].T.astype(np.float16),
                    "wrow": wpad.reshape(1, cap).astype(np.float16),
                    "brow": expert_b[e].reshape(1, H).astype(np.float16),
                }
            )

        res = run_bass_kernel_spmd(nc, in_maps, list(range(E)), trace=TRACE)
        LAST_RESULTS = res

        for e in range(E):
            rows = rows_per_e[e][s * seg:(s + 1) * seg]
            out[rows] += res.results[e]["out"][: len(rows)]
    return out, topk_idx


# revision 36
# speedup vs baseline: 1.0374x; 1.0374x over previous
"""Trainium2 Bass kernel for top-2 MoE routing (nn_MoE_29291676959130).

Strategy: expert-parallel across the 8 NeuronCores (1 expert per core).
  host (shard step) : gating matmul in float64 -> top-2 indices + softmax
                      combine weights; gather each expert's routed tokens,
                      pre-scaled by their combine weight.
  device (per core) : Y = (w*x) @ W_e^T for its <=CAP routed tokens, as a
                      float16 tiled matmul (K=2048 contraction accumulated
                      in fp32 PSUM).
  host (unshard)    : scatter-add the two expert contributions per token,
                      plus the (tiny) combine-weighted bias term.

Shapes (hardcoded): B=4096 tokens, D=2048, H=4096, E=8 experts, top-2.
"""

import numpy as np

import concourse.bass as bass
import concourse.tile as tile
from concourse import bacc, mybir
from concourse.bass_utils import run_bass_kernel_spmd

B, D, H, E, TOPK = 4096, 2048, 4096, 8, 2
P = 128
KT = D // P          # 16 k-tiles over the contraction dim
NFREE = 512          # PSUM bank free-dim (fp32)
NT = H // NFREE      # 8 n-tiles

_NC_CACHE: dict[int, object] = {}

# Set TRACE=True (e.g. from a test harness) to profile the device kernel;
# the BassKernelResults of the last run lands in LAST_RESULTS.
TRACE = False
LAST_RESULTS = None


XCHUNK = 2           # k-tiles per token/weight DMA chunk


def _build(cap: int):
    """Per-core program: out[cap,H] = xt.T @ wt  (float16 matmuls).

    K=2048 contraction accumulated in fp32 PSUM over 16 k-tile matmuls.
    Two n-tiles are paired per stationary token tile so each PE weight
    load feeds two matmuls -- this keeps the matmul stream at its
    213-216ns/matmul floor (single-n chains run ~259ns). The first
    n-pair's weight slices and the token block arrive as interleaved
    per-k chunk tiles so the first matmuls start ~equal with the DMA
    stream. (wrow/brow/zrow are vestigial and unused by the math.)
    """
    fmm = mybir.dt.float16
    f32 = mybir.dt.float32
    nc = bacc.Bacc("TRN2", target_bir_lowering=False, debug=False, num_devices=E)
    xt = nc.dram_tensor("xt", [D, cap], fmm, kind="ExternalInput").ap()
    wt = nc.dram_tensor("wt", [D, H], fmm, kind="ExternalInput").ap()
    MT = cap // P
    wrow = nc.dram_tensor("wrow", [1, cap], fmm, kind="ExternalInput").ap()
    brow = nc.dram_tensor("brow", [1, H], fmm, kind="ExternalInput").ap()
    out = nc.dram_tensor("out", [cap, H], f32, kind="ExternalOutput").ap()

    with tile.TileContext(nc) as tc:
        with (
            tc.tile_pool(name="xpool", bufs=1) as xpool,
            tc.tile_pool(name="cpool", bufs=1) as cpool,
            tc.tile_pool(name="wpool", bufs=6) as wpool,
            tc.tile_pool(name="opool", bufs=6) as opool,
            tc.tile_pool(name="pspool", bufs=6, space="PSUM") as pspool,
        ):
            xt_r = xt.rearrange("(kt p) c -> p kt c", p=P)
            wt_r = wt.rearrange("(kt p) f -> p kt f", p=P)
            # chunk boundaries over the k-tiles: the first two chunks are a
            # single k-tile so the first matmul's critical DMA path is small
            bounds = [0, 1, 2]
            while bounds[-1] < KT:
                bounds.append(min(KT, bounds[-1] + XCHUNK))
            NCH = len(bounds) - 1
            cof = []
            for c in range(NCH):
                for kk in range(bounds[c + 1] - bounds[c]):
                    cof.append((c, kk))

            # First n-pair's weight slices arrive as per-k chunks,
            # interleaved with the token chunks, so the first matmuls only
            # wait for ~2MB instead of the whole resident set.
            w0c, w1c, xc = [], [], []
            for i in range(NCH):
                lo, hi = bounds[i], bounds[i + 1]
                sz = hi - lo
                tw0 = cpool.tile([P, sz, NFREE], fmm,
                                 name=f"w0c{i}", tag=f"w0c{i}")
                nc.sync.dma_start(tw0[:], wt_r[:, lo:hi, 0:NFREE])
                w0c.append(tw0)
                tx = xpool.tile([P, sz, cap], fmm, name=f"xc{i}", tag=f"xc{i}")
                nc.sync.dma_start(tx[:], xt_r[:, lo:hi, :])
                xc.append(tx)
                tw1 = cpool.tile([P, sz, NFREE], fmm,
                                 name=f"w1c{i}", tag=f"w1c{i}")
                nc.sync.dma_start(tw1[:], wt_r[:, lo:hi, NFREE:2 * NFREE])
                w1c.append(tw1)

            wr = cpool.tile([1, cap], fmm, name="wr", tag="wr")
            nc.sync.dma_start(wr[:], wrow[:])
            br = cpool.tile([1, H], fmm, name="br", tag="br")
            nc.sync.dma_start(br[:], brow[:])
            zrow = cpool.tile([1, P], fmm, name="zrow", tag="zr")
            nc.gpsimd.memset(zrow[:], 0.0)

            # Two n-tiles share each stationary token tile: each weight load
            # feeds two matmuls (halves the PE weight-load pressure).
            for pr in range(NT // 2):
                n0 = 2 * pr
                if pr == 0:
                    wA = lambda k: w0c[cof[k][0]][:, cof[k][1], :]
                    wB = lambda k: w1c[cof[k][0]][:, cof[k][1], :]
                    # Chunk-progressive start: advance the first four
                    # m-blocks one k-chunk at a time across 8 PSUM banks,
                    # so the PE has ~28us of runnable work while the
                    # initial bulk DMA streams in (instead of one block).
                    M1 = min(4, MT)
                    accA = [pspool.tile([P, NFREE], f32, name=f"accA{m}",
                                        tag="acc0", bufs=4)
                            for m in range(M1)]
                    accB = [pspool.tile([P, NFREE], f32, name=f"accB{m}",
                                        tag="acc1", bufs=4)
                            for m in range(M1)]
                    for c in range(NCH):
                        lo, hi = bounds[c], bounds[c + 1]
                        for m in range(M1):
                            for kk in range(hi - lo):
                                k = lo + kk
                                st = xc[c][:, kk, m * P:(m + 1) * P]
                                nc.tensor.matmul(accA[m][:], st,
                                                 w0c[c][:, kk, :],
                                                 start=(k == 0),
                                                 stop=(k == KT - 1))
                                nc.tensor.matmul(accB[m][:], st,
                                                 w1c[c][:, kk, :],
                                                 start=(k == 0),
                                                 stop=(k == KT - 1))
                    for m in range(M1):
                        ot = opool.tile([P, 2 * NFREE], f32, name="ot")
                        nc.vector.tensor_copy(ot[:, 0:NFREE], accA[m][:])
                        nc.vector.tensor_copy(ot[:, NFREE:2 * NFREE], accB[m][:])
                        nc.sync.dma_start(
                            out[m * P:(m + 1) * P, 0:2 * NFREE], ot[:])
                    mstart = M1
                else:
                    mstart = 0
                    wtA = wpool.tile([P, KT, NFREE], fmm, name="wtile")
                    nc.sync.dma_start(
                        wtA[:], wt_r[:, :, n0 * NFREE:(n0 + 1) * NFREE])
                    wtB = wpool.tile([P, KT, NFREE], fmm, name="wtile")
                    nc.sync.dma_start(
                        wtB[:], wt_r[:, :, (n0 + 1) * NFREE:(n0 + 2) * NFREE])
                    wA = lambda k, t=wtA: t[:, k, :]
                    wB = lambda k, t=wtB: t[:, k, :]
                for m in range(mstart, MT):
                    acc0 = pspool.tile([P, NFREE], f32, name="acc0",
                                       tag="acc0", bufs=4)
                    acc1 = pspool.tile([P, NFREE], f32, name="acc1",
                                       tag="acc1", bufs=4)
                    for k in range(KT):
                        st = xc[cof[k][0]][:, cof[k][1], m * P:(m + 1) * P]
                        nc.tensor.matmul(acc0[:], st, wA(k),
                                         start=(k == 0), stop=(k == KT - 1))
                        nc.tensor.matmul(acc1[:], st, wB(k),
                                         start=(k == 0), stop=(k == KT - 1))
                    ot = opool.tile([P, 2 * NFREE], f32, name="ot")
                    nc.vector.tensor_copy(ot[:, 0:NFREE], acc0[:])
                    nc.vector.tensor_copy(ot[:, NFREE:2 * NFREE], acc1[:])
                    nc.sync.dma_start(
                        out[m * P:(m + 1) * P,
                            n0 * NFREE:(n0 + 2) * NFREE],
                        ot[:])
    nc.compile()
    return nc


def _get_nc(cap: int):
    if cap not in _NC_CACHE:
        _NC_CACHE[cap] = _build(cap)
    return _NC_CACHE[cap]


def _route(x, difficulty_labels, emb, gate_W, gate_b):
    """Gating in float64: returns (topk_idx int32 [B,2], probs f64 [B,2])."""
    x64 = x.astype(np.float64)
    w1 = gate_W[:, :D].astype(np.float64)          # [E, D]
    w2 = gate_W[:, D:].astype(np.float64)          # [E, D]
    table = emb.astype(np.float64) @ w2.T          # [NDIFF, E]
    logits = x64 @ w1.T + table[difficulty_labels] + gate_b.astype(np.float64)
    # jax.lax.top_k order: descending value, ties -> lower index first
    topk_idx = np.argsort(-logits, axis=1, kind="stable")[:, :TOPK]
    topw = np.take_along_axis(logits, topk_idx, axis=1)
    ex = np.exp(topw - topw.max(axis=1, keepdims=True))
    probs = ex / ex.sum(axis=1, keepdims=True)
    return topk_idx.astype(np.int32), probs


def kernel(x, difficulty_labels, emb, gate_W, gate_b, expert_W, expert_b):
    x = np.asarray(x, dtype=np.float32)
    difficulty_labels = np.asarray(difficulty_labels)
    emb = np.asarray(emb, dtype=np.float32)
    gate_W = np.asarray(gate_W, dtype=np.float32)
    gate_b = np.asarray(gate_b, dtype=np.float32)
    expert_W = np.asarray(expert_W, dtype=np.float32)
    expert_b = np.asarray(expert_b, dtype=np.float32)

    topk_idx, probs = _route(x, difficulty_labels, emb, gate_W, gate_b)

    # Per-expert routed token lists + combine weights
    rows_per_e, w_per_e = [], []
    for e in range(E):
        hit = topk_idx == e                         # [B, 2]
        mask = hit.any(axis=1)
        rows = np.nonzero(mask)[0]
        w = np.where(hit[rows, 0], probs[rows, 0], probs[rows, 1])
        rows_per_e.append(rows)
        w_per_e.append(w.astype(np.float32))

    # Pass size: normally one pass (max count ~1100 of cap 1152); the
    # multi-pass split only triggers for pathologically imbalanced routing
    # that would otherwise overflow SBUF.
    MAXCAP = 1408
    maxcnt = max(len(r) for r in rows_per_e)
    npass = max(1, -(-maxcnt // MAXCAP))
    seg = -(-maxcnt // npass)
    cap = max(P, -(-seg // P) * P)
    nc = _get_nc(cap)

    combine = np.zeros((B, E), dtype=np.float32)
    combine[np.arange(B), topk_idx[:, 0]] = probs[:, 0]
    combine[np.arange(B), topk_idx[:, 1]] = probs[:, 1]
    out = (combine @ expert_b).astype(np.float32)

    global LAST_RESULTS
    for s in range(npass):
        in_maps = []
        for e in range(E):
            rows = rows_per_e[e][s * seg:(s + 1) * seg]
            w = w_per_e[e][s * seg:(s + 1) * seg]
            xs = np.zeros((cap, D), dtype=np.float32)
            xs[: len(rows)] = x[rows] * w[:, None]
            wpad = np.zeros((cap,), dtype=np.float32)
            wpad[: len(rows)] = w
            in_maps.append(
                {
                    "xt": xs.T.astype(np.float16),
                    "wt": expert_W[e].T.astype(np.float16),
                    "wrow": wpad.reshape(1, cap).astype(np.float16),
                    "brow": expert_b[e].reshape(1, H).astype(np.float16),
                }
            )

        res = run_bass_kernel_spmd(nc, in_maps, list(range(E)), trace=TRACE)
        LAST_RESULTS = res

        for e in range(E):
            rows = rows_per_e[e][s * seg:(s + 1) * seg]
            out[rows] += res.results[e]["out"][: len(rows)]
    return out, topk_idx
